# revision 1
# baseline (speedup 1.0000x reference)
"""GATv2 layer (KNN graph, K=32, self-loops) on 8 Trainium2 NeuronCores.

Strategy (data-parallel over target nodes, 1250 rows/core):
  - similarity s[i,j] = x_i.x_j - 0.5*|x_j|^2 (order-equivalent to -dist^2 per
    row).  s[i,i] is always the row max (it equals 0.5|x_i|^2 minus nothing),
    so top-33 of s = {self} + 32 nearest neighbours: no diagonal masking.
  - PE computes s via split-bf16 (hi/lo) matmuls; the -0.5|x_j|^2 term rides in
    as a K=3 seed matmul (3-way bf16 split of the row of squared norms).
  - top-33 selection per row: per-chunk top-8 (vector.max) + index
    (max_index), then mark-and-extract rounds (max/match_replace) on the
    [128, C*8] candidate array.  Global indices come out as *values* of a
    masked index array, so no per-partition gather is ever needed.
  - h_l rows (plus a fused p = att.h_l column) live in DRAM; neighbour rows
    are fetched with gpsimd.dma_gather.
  - scores: e = 0.2*(p_j + q_n) + 0.8 * att.relu(h_l[j] + h_r[n]) which equals
    att.leaky_relu(z, 0.2); softmax over 33; weighted sum back on DVE/GPSIMD.
"""

import os
import sys

for _p in ("/opt/trn_rl_repo", os.path.expanduser("~/.axon_site/_ro/trn_rl_repo")):
    if os.path.isdir(_p) and _p not in sys.path:
        sys.path.insert(0, _p)

from contextlib import ExitStack

import ml_dtypes
import numpy as np

import concourse.bass as bass
import concourse.tile as tile
from concourse import bacc, mybir

BF16 = ml_dtypes.bfloat16

CFG = dict(
    N=10000,      # nodes
    DIN=128,      # input features (must be 128: one PE contraction)
    DOUT=256,     # output features
    KNN=32,       # neighbours (excl. self)
    NCORES=8,
    SELW=500,     # selection chunk width (top-8 per chunk)
    JCH=512,      # similarity matmul free-dim chunk
    GROW=320,     # gathered DRAM row length in f32 (256 h_l + 1 p + 63 pad)
)

NEG = -1.0e30
f32 = mybir.dt.float32
bf16 = mybir.dt.bfloat16
i16 = mybir.dt.int16
u32 = mybir.dt.uint32
FT = mybir.ActivationFunctionType
ALU = mybir.AluOpType
AX = mybir.AxisListType
P = 128


def _tile_starts(rows):
    starts = list(range(0, rows - P + 1, P))
    if starts[-1] + P < rows:
        starts.append(rows - P)
    return starts


def _split2(a):
    hi = a.astype(BF16)
    lo = (a - hi.astype(np.float32)).astype(BF16)
    return hi, lo


def _split3(a):
    out = []
    r = a.astype(np.float32)
    for _ in range(3):
        h = r.astype(BF16)
        out.append(h)
        r = r - h.astype(np.float32)
    return np.stack(out, 0)


def build_program(cfg):
    N, DOUT, KNN = cfg["N"], cfg["DOUT"], cfg["KNN"]
    SELW, JCH, GROW = cfg["SELW"], cfg["JCH"], cfg["GROW"]
    ROWS = N // cfg["NCORES"]
    SELC = (N + SELW - 1) // SELW
    assert N % SELW == 0
    K1 = KNN + 1
    NI = K1 * P                      # dma_gather index count per tile
    NC16 = NI // 16                  # wrapped index columns
    SR = (K1 + 7) // 8               # selection rounds (5 for K1=33)
    CAND = SELC * 8
    starts = _tile_starts(ROWS)
    nhl = (N + P - 1) // P

    nc = bacc.Bacc("TRN2", debug=False)

    # ---- I/O ----
    din = {}

    def inp(name, shape, dt):
        din[name] = nc.dram_tensor(name, list(shape), dt, kind="ExternalInput")
        return din[name]

    xhiT = inp("xhiT", (P, N), bf16)
    xloT = inp("xloT", (P, N), bf16)
    xhiTo = inp("xhiTo", (P, ROWS), bf16)
    xloTo = inp("xloTo", (P, ROWS), bf16)
    seed3 = inp("seed3", (P, N), bf16)     # rows 0-2: bf16 split of -0.5|x|^2
    ones3 = inp("ones3", (P, P), bf16)     # cols: 1 where row<3
    whl = inp("whl", (P, DOUT + 1), bf16)
    wll = inp("wll", (P, DOUT + 1), bf16)
    whr = inp("whr", (P, DOUT), bf16)
    wlr = inp("wlr", (P, DOUT), bf16)
    brr = inp("brr", (P, DOUT), f32)
    sgnr = inp("sgnr", (P, DOUT), f32)
    invar = inp("invar", (P, DOUT), f32)
    dpos = cfg.get("_dpos", DOUT)  # pos-sign feature count
    biasr = inp("biasr", (P, DOUT), f32)
    cw = inp("cw", (P, CAND), f32)
    ident = inp("ident", (P, P), f32)
    out_d = nc.dram_tensor("out", [ROWS, DOUT], f32, kind="ExternalOutput")
    flg_d = nc.dram_tensor("flags", [ROWS, 1], f32, kind="ExternalOutput")

    jchunks = [(a, min(JCH, N - a)) for a in range(0, N, JCH)]

    with ExitStack() as ctx:
        tc = ctx.enter_context(tile.TileContext(nc))
        cpool = ctx.enter_context(tc.tile_pool(name="consts", bufs=1))
        dpool = ctx.enter_context(tc.tile_pool(name="dram", bufs=1, space="DRAM"))
        spool = ctx.enter_context(tc.tile_pool(name="stage", bufs=2, space="DRAM"))
        psum = ctx.enter_context(tc.tile_pool(name="psum", bufs=3, space="PSUM"))
        psum_hl = ctx.enter_context(tc.tile_pool(name="psum_hl", bufs=2, space="PSUM"))
        psum_h = ctx.enter_context(tc.tile_pool(name="psum_h", bufs=1, space="PSUM"))
        hpool = ctx.enter_context(tc.tile_pool(name="hl", bufs=2))
        sp = ctx.enter_context(tc.tile_pool(name="s", bufs=1))
        selp = ctx.enter_context(tc.tile_pool(name="sel", bufs=1))
        gp = ctx.enter_context(tc.tile_pool(name="g", bufs=1))
        zp = ctx.enter_context(tc.tile_pool(name="z", bufs=1))
        smp = ctx.enter_context(tc.tile_pool(name="small", bufs=2))
        op = ctx.enter_context(tc.tile_pool(name="outs", bufs=2))

        # ---- load constants ----
        def load(t, w=None):
            tl = cpool.tile(list(t.shape), t.dtype, tag=t.name)
            nc.sync.dma_start(tl[:], t.ap())
            return tl

        xhiT_s, xloT_s = load(xhiT), load(xloT)
        xhiTo_s, xloTo_s = load(xhiTo), load(xloTo)
        seed3_s, ones3_s = load(seed3), load(ones3)
        whl_s, wll_s, whr_s, wlr_s = load(whl), load(wll), load(whr), load(wlr)
        brr_s, sgnr_s, invar_s, biasr_s = load(brr), load(sgnr), load(invar), load(biasr)
        cw_s, ident_s = load(cw), load(ident)

        # ---- phase B: h_l (+p column) for all nodes -> DRAM ----
        hl_d = dpool.tile([N, GROW], f32)
        for i in range(nhl):
            w_ = min(P, N - i * P)
            ps = psum_hl.tile([P, DOUT + 1], f32, tag="hlp")
            lo = i * P
            nc.tensor.matmul(ps[:w_], xhiT_s[:, lo:lo + w_], whl_s[:], start=True, stop=False)
            nc.tensor.matmul(ps[:w_], xhiT_s[:, lo:lo + w_], wll_s[:], start=False, stop=False)
            nc.tensor.matmul(ps[:w_], xloT_s[:, lo:lo + w_], whl_s[:], start=False, stop=True)
            hb = hpool.tile([P, GROW], f32, tag="hb")
            nc.gpsimd.memset(hb[:, DOUT + 1:], 0.0)
            nc.scalar.activation(hb[:w_, :DOUT + 1], ps[:w_], FT.Copy)
            nc.sync.dma_start(hl_d[lo:lo + w_, :], hb[:w_])

        # ---- phase C: per 128-row tile ----
        for ts_ in starts:
            # h_r for this tile (+ q = att.h_r)
            pr = psum_h.tile([P, DOUT], f32, tag="hrp")
            nc.tensor.matmul(pr[:], xhiTo_s[:, ts_:ts_ + P], whr_s[:], start=True, stop=False)
            nc.tensor.matmul(pr[:], xhiTo_s[:, ts_:ts_ + P], wlr_s[:], start=False, stop=False)
            nc.tensor.matmul(pr[:], xloTo_s[:, ts_:ts_ + P], whr_s[:], start=False, stop=True)
            hr = smp.tile([P, DOUT], f32, tag="hr")
            nc.vector.tensor_add(hr[:], pr[:], brr_s[:])
            q02 = smp.tile([P, 1], f32, tag="q02")
            tscr = smp.tile([P, DOUT], f32, tag="tscr", bufs=1)
            nc.vector.tensor_mul(tscr[:], hr[:], sgnr_s[:])
            nc.vector.reduce_sum(q02[:], tscr[:], axis=AX.X)
            nc.vector.tensor_scalar_mul(q02[:], q02[:], 0.2)

            # similarity row-block s = x_i.x_j - 0.5|x_j|^2
            s_sb = sp.tile([P, N], f32, tag="s")
            for (a, w_) in jchunks:
                ps = psum.tile([P, w_], f32, tag="sp")
                nc.tensor.matmul(ps[:], ones3_s[:, :P], seed3_s[:, a:a + w_], start=True, stop=False)
                nc.tensor.matmul(ps[:], xhiTo_s[:, ts_:ts_ + P], xhiT_s[:, a:a + w_], start=False, stop=False)
                nc.tensor.matmul(ps[:], xhiTo_s[:, ts_:ts_ + P], xloT_s[:, a:a + w_], start=False, stop=False)
                nc.tensor.matmul(ps[:], xloTo_s[:, ts_:ts_ + P], xhiT_s[:, a:a + w_], start=False, stop=True)
                nc.scalar.activation(s_sb[:, a:a + w_], ps[:], FT.Copy)

            # --- selection: per-chunk top-8 + indices ---
            v8 = selp.tile([P, CAND], f32, tag="v8")
            l8 = selp.tile([P, CAND], u32, tag="l8")
            for c in range(SELC):
                nc.vector.max(v8[:, 8 * c:8 * c + 8], s_sb[:, SELW * c:SELW * (c + 1)])
                nc.vector.max_index(l8[:, 8 * c:8 * c + 8], v8[:, 8 * c:8 * c + 8],
                                    s_sb[:, SELW * c:SELW * (c + 1)])
            glp1 = selp.tile([P, CAND], f32, tag="glp1")
            nc.vector.tensor_copy(glp1[:], l8[:])
            nc.vector.tensor_add(glp1[:], glp1[:], cw_s[:])  # global_idx + 1

            # --- rounds on values: mark top-33 with NEG ---
            candA = selp.tile([P, CAND], f32, tag="candA")
            candB = selp.tile([P, CAND], f32, tag="candB")
            cur = v8
            for r in range(SR - 1):
                m8 = smp.tile([P, 8], f32, tag=f"m8_{r % 2}")
                nc.vector.max(m8[:], cur[:])
                nxt = candA if r % 2 == 0 else candB
                nc.vector.match_replace(nxt[:], m8[:], cur[:], NEG)
                cur = nxt
            nlast = K1 - 8 * (SR - 1)          # 1 for K1=33
            m5 = smp.tile([P, 8], f32, tag="m5")
            nc.vector.max(m5[:], cur[:])
            vx8 = smp.tile([P, 8], f32, tag="vx8")
            nc.vector.tensor_copy(vx8[:], m5[:, nlast - 1:nlast].broadcast_to((P, 8)))
            fin = candB if cur is candA else candA
            nc.vector.match_replace(fin[:], vx8[:], cur[:], NEG)

            # --- mask -> masked global indices -> extract as values ---
            mask = selp.tile([P, CAND], f32, tag="mask")
            nc.vector.tensor_scalar(mask[:], fin[:], -1.0e29, None, op0=ALU.is_le)
            midxA = selp.tile([P, CAND], f32, tag="midxA")
            nc.vector.tensor_mul(midxA[:], glp1[:], mask[:])
            midxB = selp.tile([P, CAND], f32, tag="midxB")
            nc.vector.tensor_scalar_add(midxB[:], midxA[:], -1.0)

            # --- risky-row flags: (a) possible chunk overflow, (b) tiny
            #     rank-33/34 margin, (c) mark-count != K1.  Host recomputes. ---
            flg = smp.tile([P, 1], f32, tag="flg")
            f40 = smp.tile([P, SELC], f32, tag="f40")
            v8l = v8[:].rearrange("p (c e) -> p c e", e=8)[:, :, 7]
            nc.vector.tensor_scalar(f40[:], v8l, m5[:, 0:1], None, op0=ALU.is_ge)
            nc.vector.tensor_reduce(flg[:], f40[:], axis=AX.X, op=ALU.max)
            fm = smp.tile([P, 1], f32, tag="fm")
            nc.vector.tensor_sub(fm[:], m5[:, 0:1], m5[:, 1:2])
            nc.vector.tensor_scalar(fm[:], fm[:], 5.0e-4, None, op0=ALU.is_lt)
            nc.vector.tensor_add(flg[:], flg[:], fm[:])
            fc = smp.tile([P, 1], f32, tag="fc")
            nc.vector.tensor_reduce(fc[:], mask[:], axis=AX.X, op=ALU.add)
            nc.vector.tensor_scalar(fc[:], fc[:], float(K1), None, op0=ALU.subtract)
            nc.vector.tensor_scalar(fc[:], fc[:], 0.0, None, op0=ALU.not_equal)
            nc.vector.tensor_add(flg[:], flg[:], fc[:])
            nc.sync.dma_start(flg_d.ap()[ts_:ts_ + P, :], flg[:])

            idxf = smp.tile([P, 8 * SR], f32, tag="idxf")
            cur = midxB
            nxt = midxA
            for r in range(SR):
                nc.vector.max(idxf[:, 8 * r:8 * r + 8], cur[:])
                if r < SR - 1:
                    nc.vector.match_replace(nxt[:], idxf[:, 8 * r:8 * r + 8], cur[:], -1.0)
                    cur, nxt = nxt, cur
            K1p = K1 + (-K1) % 2                      # 34: xbar needs cols%16==0
            NC16p = K1p * 8
            idxc = smp.tile([P, K1p], f32, tag="idxc", bufs=1)
            nc.vector.tensor_scalar_max(idxc[:, :K1], idxf[:, :K1], 0.0)
            nc.vector.tensor_copy(idxc[:, K1:], idxc[:, :K1p - K1])

            # --- wrap indices: PE-transpose to [K1p, P], flat store (i-order),
            #     then xbar-transpose DMAs build the [16, NC16p] wrapped form ---
            pst = psum_h.tile([K1p, P], f32, tag="pst")
            nc.tensor.transpose(pst[:], idxc[:], ident_s[:])
            tc_f = smp.tile([K1p, P], f32, tag="tc_f", bufs=1)
            nc.scalar.activation(tc_f[:], pst[:], FT.Copy)
            tc_i = smp.tile([K1p, P], i16, tag="tc_i", bufs=1)
            nc.vector.tensor_copy(tc_i[:], tc_f[:])
            stg = spool.tile([K1p * P], i16, tag="stg")
            nc.sync.dma_start(stg[:].rearrange("(c p) -> c p", c=K1p), tc_i[:])
            idx16 = smp.tile([P, NC16p], i16, tag="idx16")
            src16 = stg[:].rearrange("(col p16) -> p16 col", p16=16)
            nc.sync.dma_start(idx16[0:16, :], src16)
            try:
                nc.sync.dma_start(
                    idx16[16:, :].rearrange("(r p) c -> r p c", r=7),
                    idx16[0:16, :].broadcast_to((7, 16, NC16p)))
            except Exception:
                for r in range(1, 8):
                    nc.sync.dma_start(idx16[16 * r:16 * (r + 1), :], idx16[0:16, :])
            g = gp.tile([P, K1, GROW], f32, tag="g")
            for c0, c1 in ((0, 8), (8, 16), (16, 24), (24, 32), (32, K1)):
                ni = (c1 - c0) * P
                nc.gpsimd.dma_gather(g[:, c0:c1, :], hl_d[:],
                                     idx16[:, c0 * 8:c1 * 8],
                                     num_idxs=ni, num_idxs_reg=ni,
                                     elem_size=GROW)

            # --- scores ---
            u = zp.tile([P, K1, DOUT], f32, tag="u")
            nc.vector.tensor_add(
                u[:], g[:, :, :DOUT],
                hr[:].rearrange("p (o d) -> p o d", o=1).broadcast_to((P, K1, DOUT)))
            nc.scalar.activation(u[:].rearrange("p k d -> p (k d)"),
                                 u[:].rearrange("p k d -> p (k d)"), FT.Relu)
            e8 = smp.tile([P, K1], f32, tag="e8")
            if 0 < dpos < DOUT:
                e8n = smp.tile([P, K1], f32, tag="e8n")
                nc.vector.tensor_reduce(e8[:], u[:, :, :dpos], axis=AX.X, op=ALU.add)
                nc.vector.tensor_reduce(e8n[:], u[:, :, dpos:], axis=AX.X, op=ALU.add)
                nc.vector.tensor_sub(e8[:], e8[:], e8n[:])
            else:
                nc.vector.tensor_reduce(e8[:], u[:], axis=AX.X, op=ALU.add)
                if dpos == 0:
                    nc.vector.tensor_scalar_mul(e8[:], e8[:], -1.0)
            # e = 0.8*e8 + 0.2*p_g + 0.2*q
            ee = smp.tile([P, K1], f32, tag="ee")
            nc.vector.tensor_scalar(ee[:], g[:, :, DOUT], 0.2, q02[:], op0=ALU.mult, op1=ALU.add)
            nc.vector.tensor_scalar(e8[:], e8[:], 0.8, None, op0=ALU.mult)
            nc.vector.tensor_add(ee[:], ee[:], e8[:])
            # softmax over 33
            mx = smp.tile([P, 1], f32, tag="mx")
            nc.vector.reduce_max(mx[:], ee[:], axis=AX.X)
            nc.vector.tensor_scalar_mul(mx[:], mx[:], -1.0)
            ex = smp.tile([P, K1], f32, tag="ex")
            nc.scalar.activation(ex[:], ee[:], FT.Exp, bias=mx[:], scale=1.0)
            sm = smp.tile([P, 1], f32, tag="sm")
            nc.vector.reduce_sum(sm[:], ex[:], axis=AX.X)
            nc.vector.reciprocal(sm[:], sm[:])
            al = smp.tile([P, K1], f32, tag="al")
            nc.vector.tensor_scalar_mul(al[:], ex[:], sm[:])
            # weighted sum
            nc.vector.tensor_mul(
                u[:], g[:, :, :DOUT],
                al[:].rearrange("p (k o) -> p k o", o=1).broadcast_to((P, K1, DOUT)))
            ob = op.tile([P, DOUT], f32, tag="ob")
            nc.vector.tensor_reduce(ob[:], u[:].rearrange("p k d -> p d k"),
                                    axis=AX.X, op=ALU.add)
            nc.vector.tensor_mul(ob[:], ob[:], invar_s[:])
            nc.vector.tensor_add(ob[:], ob[:], biasr_s[:])
            nc.sync.dma_start(out_d.ap()[ts_:ts_ + P, :], ob[:])

    nc.compile()
    return nc


def host_prep(x, W_l, b_l, W_r, b_r, att, bias, cfg):
    """Build the per-core input maps (numpy only; cheap O(N*D) work)."""
    N, DOUT = cfg["N"], cfg["DOUT"]
    ROWS = N // cfg["NCORES"]
    SELC = N // cfg["SELW"]
    CAND = SELC * 8

    x = np.asarray(x, np.float32)
    xhi = x.astype(BF16)
    xlo = (x - xhi.astype(np.float32)).astype(BF16)
    xhiT = np.ascontiguousarray(xhi.T)
    xloT = np.ascontiguousarray(xlo.T)
    sq = (x.astype(np.float64) ** 2).sum(1)
    seed3 = np.zeros((P, N), BF16)
    seed3[:3] = _split3(-0.5 * sq)
    ones3 = np.zeros((P, P), BF16)
    ones3[:3] = 1

    att = np.asarray(att, np.float32)
    # permute output features: att>0 block first; fold |att| into weights.
    perm = np.argsort(att <= 0, kind="stable")
    aperm = att[perm]
    aabs = np.maximum(np.abs(aperm), 1e-30)
    wp = (W_l.astype(np.float64) @ att.astype(np.float64)).astype(np.float32)
    wle = np.concatenate([W_l[:, perm] * aabs[None, :], wp[:, None]], 1)
    whl, wll = _split2(wle)
    whr, wlr = _split2(np.asarray(W_r, np.float32)[:, perm] * aabs[None, :])
    bl = np.asarray(b_l, np.float32)
    brr = np.tile(((np.asarray(b_r, np.float32) + bl)[perm]
                   * aabs)[None, :], (P, 1))
    sgnr = np.tile(np.sign(aperm)[None, :], (P, 1)).astype(np.float32)
    invar = np.tile((1.0 / aabs)[None, :], (P, 1)).astype(np.float32)
    biasr = np.tile((np.asarray(bias, np.float32)
                     + np.asarray(b_l, np.float32))[perm][None, :], (P, 1))
    cwrow = (np.arange(CAND) // 8 * cfg["SELW"] + 1).astype(np.float32)
    cw = np.tile(cwrow[None, :], (P, 1))
    ident = np.eye(P, dtype=np.float32)

    shared = dict(xhiT=xhiT, xloT=xloT, seed3=seed3, ones3=ones3,
                  whl=whl, wll=wll, whr=whr, wlr=wlr, brr=brr,
                  sgnr=sgnr, invar=invar, biasr=biasr, cw=cw, ident=ident)
    host_prep.last_perm = perm
    host_prep.last_dpos = int((aperm > 0).sum())
    in_maps = []
    for c in range(cfg["NCORES"]):
        R = c * ROWS
        m = dict(shared)
        m["xhiTo"] = np.ascontiguousarray(xhiT[:, R:R + ROWS])
        m["xloTo"] = np.ascontiguousarray(xloT[:, R:R + ROWS])
        in_maps.append(m)
    return in_maps


_PROG_CACHE = {}


def _get_program(dpos):
    if dpos not in _PROG_CACHE:
        cfg = dict(CFG)
        cfg["_dpos"] = dpos
        _PROG_CACHE[dpos] = build_program(cfg)
    return _PROG_CACHE[dpos]


def kernel(x, W_l, b_l, W_r, b_r, att, bias, _trace=False):
    from concourse import bass_utils

    cfg = CFG
    in_maps = host_prep(x, W_l, b_l, W_r, b_r, att, bias, cfg)
    perm = host_prep.last_perm
    nc = _get_program(host_prep.last_dpos)
    try:
        res = bass_utils.run_bass_kernel_spmd(
            nc, in_maps, core_ids=list(range(cfg["NCORES"])), trace=_trace)
    except ModuleNotFoundError:
        res = bass_utils.run_bass_kernel_spmd(
            nc, in_maps, core_ids=list(range(cfg["NCORES"])), trace=False)
    outp = np.concatenate([r["out"] for r in res.results], 0)
    out = np.empty_like(outp)
    out[:, perm] = outp
    kernel.last_exec_time_ns = res.exec_time_ns
    flags = np.concatenate([r["flags"][:, 0] for r in res.results], 0)
    rows = np.where(flags != 0.0)[0]
    if rows.size:
        _patch_rows(out, rows, x, W_l, b_l, W_r, b_r, att, bias, cfg)
    return out.astype(np.float32)


def _patch_rows(out, rows, x, W_l, b_l, W_r, b_r, att, bias, cfg):
    """Exact (float64) recompute of flagged rows (near-ties / rare overflow)."""
    K = cfg["KNN"]
    x64 = np.asarray(x, np.float64)
    sq = (x64 * x64).sum(1)
    h_l = x64 @ np.asarray(W_l, np.float64) + np.asarray(b_l, np.float64)
    att64 = np.asarray(att, np.float64)
    W_r64 = np.asarray(W_r, np.float64)
    for r in rows:
        d = sq + sq[r] - 2.0 * (x64 @ x64[r])
        d[r] = np.inf
        nbr = np.argpartition(d, K)[:K]
        src = np.concatenate([nbr, [r]])
        h_r = x64[r] @ W_r64 + np.asarray(b_r, np.float64)
        z = h_l[src] + h_r[None, :]
        lr = np.where(z > 0, z, 0.2 * z)
        e = lr @ att64
        e = e - e.max()
        a = np.exp(e)
        a /= a.sum()
        out[r] = (a @ h_l[src] + np.asarray(bias, np.float64)).astype(np.float32)

